# revision 1
# baseline (speedup 1.0000x reference)
"""Multi-head self-attention (B=8, E=512, heads=8, S=1024) on 8 trn2 cores.

Sharding: data-parallel over batch — core b computes batch element b end to
end (no collectives). Weights are replicated; Wq/Wk/Wv/Wo are passed
pre-transposed ([in_ch, out_ch]) so their natural DRAM layout matches the
stationary-operand layout the PE wants.

Per-core pipeline (everything stays in "transposed" channel-major layout so
no attention-matrix transposes are ever needed, and the final output is
already channels-first as the module requires):
  1. xs [S, C] -> xsT [C, S] via PE transposes (32x 128x128 blocks).
  2. qT = WqT.T @ xsT, kT likewise (channel-major); v = xsT.T @ WvT
     (token-major) — all with K-accumulation in PSUM.
  3. Per head pair and query-half (scores psum double-buffered so exp(t)
     overlaps scores(t+1)): scoresT[t2, t1] = kT.T @ qT via row-packed K=64
     matmuls (two heads concurrent on disjoint PE row groups), exp via ACT
     (scale=1/8 folded in; max-subtraction unnecessary: |scores| <= ~9.5,
     verified on host), ctx^T[dv, t1] = v_aug.T @ E accumulated over key
     blocks — v carries an interleaved ones column per head so psum row 64
     accumulates the softmax denominator in the same group. Next-pair q/k
     projection groups are interleaved into the ACT-bound loop.
  4. Normalize: reciprocal of the denominator row, broadcast to the 64 dv
     partitions via a DRAM-bounce DMA (zero-step partition source; NB
     gpsimd.partition_broadcast corrupts data on real HW), DVE multiply
     into zT [C, S].
  5. outT = WoT.T @ zT + bo -> DRAM [C, S] (= channels-first output layout).
"""

import numpy as np
from contextlib import ExitStack

import concourse.bass as bass
import concourse.mybir as mybir
import concourse.tile as tile
from concourse import bacc
from concourse.bass_utils import run_bass_kernel_spmd

B = 8
C = 512
HH = 32
WW = 32
S = HH * WW            # 1024
HEADS = 8
HD = C // HEADS        # 64
CB = C // 128          # 4 channel blocks
TB = S // 128          # 8 token blocks
CHUNK = 512            # fp32 moving-operand max
NCH = S // CHUNK       # 2
F32 = mybir.dt.float32
MM_DT = mybir.dt.float32r  # fp32r: full-rate PE at N>=256; fall back to float32 if accuracy demands

EXP = mybir.ActivationFunctionType.Exp
ADD = mybir.AluOpType.add
MULT = mybir.AluOpType.mult




def build_nc(reps=1):
    nc = bacc.Bacc()
    xs_d = nc.declare_dram_parameter("xs", [S, C], MM_DT, isOutput=False)
    w_d = {
        n: nc.declare_dram_parameter(n, [C, C], MM_DT, isOutput=False)
        for n in ("wqT", "wkT", "wvT", "woT")
    }
    b_d = {
        n: nc.declare_dram_parameter(n, [C, 1], F32, isOutput=False)
        for n in ("bq", "bk", "bv", "bo")
    }
    ident_d = nc.declare_dram_parameter("ident", [128, 128], MM_DT, isOutput=False)
    bvbc_d = nc.declare_dram_parameter("bv_bc", [128, C], F32, isOutput=False)
    vones_d = nc.declare_dram_parameter("vones", [128, HEADS], MM_DT, isOutput=False)
    out_d = nc.declare_dram_parameter("out", [C, S], F32, isOutput=True)

    with tile.TileContext(nc) as tc, ExitStack() as ctx:
        pools = _make_pools(ctx, tc)
        for _ in range(reps):
            _emit(pools, tc, nc, xs_d, w_d, b_d, ident_d, bvbc_d, vones_d, out_d)
    nc.compile()
    return nc


def _make_pools(ctx, tc):
    return {
        "sb": ctx.enter_context(tc.tile_pool(name="sb", bufs=1)),
        "ps": ctx.enter_context(tc.tile_pool(name="ps", bufs=2, space="PSUM")),
        "ep": ctx.enter_context(tc.tile_pool(name="ep", bufs=6)),
        "np": ctx.enter_context(tc.tile_pool(name="npool", bufs=6)),
        "dr": ctx.enter_context(tc.tile_pool(name="drpool", bufs=4, space="DRAM")),
    }


def _emit(pools, tc, nc, xs_d, w_d, b_d, ident_d, bvbc_d, vones_d, out_d):
    # PSUM budget (8 banks): "sc" [128,1024] x2 = 4 banks (scores pipeline +
    # general matmul groups), "cx" [65,512] x4 = 4 banks (ctx accumulators,
    # two live + two draining through normalization).
    sb = pools["sb"]
    ps = pools["ps"]
    ep = pools["ep"]
    np_pool = pools["np"]
    dr_pool = pools["dr"]

    def sc_tile(shape=(128, 1024)):
        return ps.tile(list(shape), F32, tag="sc", bufs=2, name="sc")

    def cx_tile():
        return ps.tile([65, 512], F32, tag="cx", bufs=4, name="cx")

    # ---- input DMAs (xs first: transposes gate everything) ----
    ident = sb.tile([128, 128], MM_DT, tag="ident", name="ident")
    nc.sync.dma_start(ident, ident_d[:, :])
    xs = []
    for i in range(TB):
        t = sb.tile([128, C], MM_DT, tag=f"xs{i}", name=f"xs{i}")
        nc.sync.dma_start(t, xs_d[i * 128:(i + 1) * 128, :])
        xs.append(t)
    w = {}
    for n in ("wvT", "wqT", "wkT"):
        w[n] = []
        for j in range(CB):
            t = sb.tile([128, C], MM_DT, tag=f"{n}{j}", name=f"{n}{j}")
            nc.sync.dma_start(t, w_d[n][j * 128:(j + 1) * 128, :])
            w[n].append(t)
    bv_bc = sb.tile([128, C], F32, tag="bv_bc", name="bv_bc")
    nc.sync.dma_start(bv_bc, bvbc_d[:, :])
    bias = {}
    for n in ("bq", "bk", "bo"):
        bias[n] = []
        for j in range(CB):
            t = sb.tile([128, 1], F32, tag=f"{n}{j}", name=f"{n}{j}")
            nc.sync.dma_start(t, b_d[n][j * 128:(j + 1) * 128, :])
            bias[n].append(t)

    # ---- xsT = xs.T (PE transpose, 128x128 blocks) ----
    xsT = [sb.tile([128, S], MM_DT, tag=f"xsT{j}", name=f"xsT{j}") for j in range(CB)]
    for i in range(TB):
        for j in range(CB):
            pt = ps.tile([128, 1024], MM_DT, tag="sc", bufs=2, name="sc")
            nc.tensor.transpose(pt[:, :128], xs[i][:, j * 128:(j + 1) * 128], ident)
            if (i + j) % 2 == 0:
                nc.scalar.copy(xsT[j][:, i * 128:(i + 1) * 128], pt[:, :128])
            else:
                nc.vector.tensor_copy(xsT[j][:, i * 128:(i + 1) * 128], pt[:, :128])

    # ---- v (token-major, interleaved ones column per head) ----
    # v[i] is [128, 8*65]; head h at cols h*65..h*65+63, col h*65+64 = 1.0 so
    # the ctx matmul's stationary [t2, 65] yields ctx rows 0-63 AND the
    # softmax denominator in row 64 of one accumulation group.
    v = [sb.tile([128, HEADS * (HD + 1)], MM_DT, tag=f"v{i}", name=f"v{i}")
         for i in range(TB)]
    for i in range(TB):
        v3 = v[i].rearrange("p (h d) -> p h d", d=HD + 1)
        nc.sync.dma_start(v3[:, :, HD:HD + 1], vones_d[:, :].unsqueeze(2))
        pt = sc_tile() if i % 2 == 0 else ps.tile([128, 512], F32, tag="cx", bufs=4, name="cx")
        for j in range(CB):
            nc.tensor.matmul(
                pt[:128, 0:512],
                lhsT=xsT[j][:, i * 128:(i + 1) * 128],
                rhs=w["wvT"][j],
                start=(j == 0),
                stop=(j == CB - 1),
            )
        nc.vector.tensor_tensor(
            v3[:, :, 0:HD],
            pt[:, 0:512].rearrange("p (h d) -> p h d", d=HD),
            bv_bc.rearrange("p (h d) -> p h d", d=HD),
            ADD,
        )



    # ---- q/k projections for one head pair (channel-major) ----
    qT = [sb.tile([128, S], MM_DT, tag=f"qT{m}", name=f"qT{m}") for m in range(CB)]
    kT = [sb.tile([128, S], MM_DT, tag=f"kT{m}", name=f"kT{m}") for m in range(CB)]

    def qk_group(wn, bn, dest, m, n):
        pt = sc_tile()
        for j in range(CB):
            nc.tensor.matmul(
                pt[:, 0:512],
                lhsT=w[wn][j][:, m * 128:(m + 1) * 128],
                rhs=xsT[j][:, n * CHUNK:(n + 1) * CHUNK],
                start=(j == 0),
                stop=(j == CB - 1),
            )
        nc.vector.tensor_scalar_add(
            dest[m][:, n * CHUNK:(n + 1) * CHUNK], pt[:, 0:512], bias[bn][m]
        )

    def qk_groups_for(m):
        return [
            (wn, bn, dest, m, n)
            for wn, bn, dest in (("wqT", "bq", qT), ("wkT", "bk", kT))
            for n in range(NCH)
        ]

    for g in qk_groups_for(0):
        qk_group(*g)

    # ---- attention: pair hp, query-half n; scores psum double-buffered so
    # exp(t2) overlaps scores(t2+1); ctx accumulates in [65,512] banks ----
    zT = [sb.tile([128, S], MM_DT, tag=f"zT{hp}", name=f"zT{hp}") for hp in range(CB)]
    for hp in range(CB):
        qh, kh = qT[hp], kT[hp]
        pending_qk = qk_groups_for(hp + 1) if hp + 1 < CB else []
        for n in range(NCH):
            cps = [cx_tile(), cx_tile()]   # head A, head B

            def ctx_mms(t2, E):
                for half in range(2):
                    h = 2 * hp + half
                    nc.tensor.matmul(
                        cps[half][0:HD + 1, :],
                        lhsT=v[t2][:, h * (HD + 1):(h + 1) * (HD + 1)],
                        rhs=E[:, half * 512:(half + 1) * 512],
                        start=(t2 == 0), stop=(t2 == TB - 1),
                    )

            for t2 in range(TB):
                sc = sc_tile()
                nc.tensor.matmul(
                    sc[:, 0:512],
                    lhsT=kh[0:64, t2 * 128:(t2 + 1) * 128],
                    rhs=qh[0:64, n * CHUNK:(n + 1) * CHUNK],
                    start=True, stop=True,
                    tile_position=(0, 0),
                )
                nc.tensor.matmul(
                    sc[:, 512:1024],
                    lhsT=kh[64:128, t2 * 128:(t2 + 1) * 128],
                    rhs=qh[64:128, n * CHUNK:(n + 1) * CHUNK],
                    start=True, stop=True,
                    tile_position=(64, 0),
                )
                E = ep.tile([128, 1024], MM_DT, tag="E", name="E")
                nc.scalar.activation(E, sc, EXP, scale=1.0 / np.sqrt(HD))
                ctx_mms(t2, E)
                if pending_qk and n == 0 and t2 in (3, 4, 5, 6):
                    qk_group(*pending_qk.pop(0))
            # normalization for this (pair, half): 1/denominator at partition
            # 64, DRAM-bounce broadcast down to the 64 dv partitions, multiply.
            for half in range(2):
                cp = cps[half]
                rs = np_pool.tile([65, 512], F32, tag="rs", name="rs")
                nc.vector.reciprocal(rs[64:65, :], cp[64:65, :])
                r_dram = dr_pool.tile([1, 512], F32, tag="r_dram", name="r_dram")
                nc.sync.dma_start(r_dram, rs[64:65, :])
                rb = np_pool.tile([64, 512], F32, tag="rb", name="rb")
                nc.sync.dma_start(rb, r_dram[0:1, :].partition_broadcast(64))
                nc.vector.tensor_tensor(
                    zT[hp][half * 64:(half + 1) * 64, n * CHUNK:(n + 1) * CHUNK],
                    cp[0:64, :],
                    rb,
                    MULT,
                )
        for g in pending_qk:
            qk_group(*g)

    # ---- output projection (Wo loaded late: keeps early DMA bandwidth
    # for xs/Wv, and the transfer hides under the attention phase) ----
    w["woT"] = []
    for j in range(CB):
        t = sb.tile([128, C], MM_DT, tag=f"woT{j}", name=f"woT{j}")
        nc.sync.dma_start(t, w_d["woT"][j * 128:(j + 1) * 128, :])
        w["woT"].append(t)
    outT = [sb.tile([128, S], F32, tag=f"outT{m}", name=f"outT{m}") for m in range(CB)]
    for m in range(CB):
        for n in range(NCH):
            pt = sc_tile()
            for j in range(CB):
                nc.tensor.matmul(
                    pt[:, 0:512],
                    lhsT=w["woT"][j][:, m * 128:(m + 1) * 128],
                    rhs=zT[j][:, n * CHUNK:(n + 1) * CHUNK],
                    start=(j == 0),
                    stop=(j == CB - 1),
                )
            nc.vector.tensor_scalar_add(
                outT[m][:, n * CHUNK:(n + 1) * CHUNK], pt[:, 0:512], bias["bo"][m]
            )
            nc.sync.dma_start(
                out_d[m * 128:(m + 1) * 128, n * CHUNK:(n + 1) * CHUNK],
                outT[m][:, n * CHUNK:(n + 1) * CHUNK],
            )


_NC_CACHE = None


def _get_nc():
    global _NC_CACHE
    if _NC_CACHE is None:
        _NC_CACHE = build_nc()
    return _NC_CACHE


def _in_maps(x, Wq, bq, Wk, bk, Wv, bv, Wo, bo):
    x = np.ascontiguousarray(np.asarray(x, np.float32))
    base = {
        "ident": np.eye(128, dtype=np.float32),
        "vones": np.ones((128, HEADS), np.float32),
        "bv_bc": np.ascontiguousarray(
            np.broadcast_to(np.asarray(bv, np.float32), (128, C))
        ),
        "wqT": np.ascontiguousarray(np.asarray(Wq, np.float32).T),
        "wkT": np.ascontiguousarray(np.asarray(Wk, np.float32).T),
        "wvT": np.ascontiguousarray(np.asarray(Wv, np.float32).T),
        "woT": np.ascontiguousarray(np.asarray(Wo, np.float32).T),
        "bq": np.asarray(bq, np.float32).reshape(C, 1),
        "bk": np.asarray(bk, np.float32).reshape(C, 1),
        "bv": np.asarray(bv, np.float32).reshape(C, 1),
        "bo": np.asarray(bo, np.float32).reshape(C, 1),
    }
    return [dict(base, xs=x[b].reshape(S, C)) for b in range(B)]


def _run(trace=False, **inputs):
    nc = _get_nc()
    maps = _in_maps(**inputs)
    res = run_bass_kernel_spmd(nc, maps, core_ids=list(range(B)), trace=trace)
    out = np.stack(
        [np.asarray(res.results[b]["out"]).reshape(C, HH, WW) for b in range(B)]
    ).astype(np.float32)
    return out, res


def kernel(**inputs):
    out, _ = _run(trace=False, **inputs)
    return out


def _make_runner(reps=1, **inputs):
    """Benchmark helper (test-only): one jitted 8-core callable, reusable
    across calls so per-execution wall time can be measured without
    re-tracing. Mirrors bass2jax.run_bass_via_pjrt's multi-core path."""
    import jax
    import numpy as _np
    from jax.sharding import Mesh, PartitionSpec
    from jax.experimental.shard_map import shard_map
    from concourse import bass2jax, mybir as _mb

    bass2jax.install_neuronx_cc_hook()
    nc = _get_nc() if reps == 1 else build_nc(reps)
    maps = _in_maps(**inputs)

    partition_name = (
        nc.partition_id_tensor.name if nc.partition_id_tensor else None
    )
    in_names, out_names, out_avals, zero_outs = [], [], [], []
    for alloc in nc.m.functions[0].allocations:
        if not isinstance(alloc, _mb.MemoryLocationSet):
            continue
        name = alloc.memorylocations[0].name
        if alloc.kind == "ExternalInput":
            if name != partition_name:
                in_names.append(name)
        elif alloc.kind == "ExternalOutput":
            shape = tuple(alloc.tensor_shape)
            dtype = _mb.dt.np(alloc.dtype)
            out_names.append(name)
            out_avals.append(jax.core.ShapedArray(shape, dtype))
            zero_outs.append(_np.zeros(shape, dtype))
    n_params = len(in_names)
    all_in_names = list(in_names) + list(out_names)
    if partition_name is not None:
        all_in_names.append(partition_name)

    def _body(*args):
        operands = list(args)
        if partition_name is not None:
            operands.append(bass2jax.partition_id_tensor())
        outs = bass2jax._bass_exec_p.bind(
            *operands,
            out_avals=tuple(out_avals),
            in_names=tuple(all_in_names),
            out_names=tuple(out_names),
            lowering_input_output_aliases=(),
            sim_require_finite=True,
            sim_require_nnan=True,
            nc=nc,
        )
        return tuple(outs)

    devices = jax.devices()[:B]
    mesh = Mesh(_np.asarray(devices), ("core",))
    n_outs = len(out_avals)
    sharded = jax.jit(
        shard_map(
            _body,
            mesh=mesh,
            in_specs=(PartitionSpec("core"),) * (n_params + n_outs),
            out_specs=(PartitionSpec("core"),) * n_outs,
            check_rep=False,
        ),
        keep_unused=True,
    )
    sh = jax.sharding.NamedSharding(mesh, PartitionSpec("core"))
    concat_in = [
        jax.device_put(
            _np.concatenate([_np.asarray(maps[c][n]) for c in range(B)], axis=0), sh
        )
        for n in in_names
    ]
    concat_zeros = [
        jax.device_put(_np.zeros((B * z.shape[0], *z.shape[1:]), z.dtype), sh)
        for z in zero_outs
    ]

    def run():
        return sharded(*concat_in, *concat_zeros)

    return run

